# revision 1
# baseline (speedup 1.0000x reference)
"""FastGTN forward on 8 Trainium2 NeuronCores (Bass/Tile).

Strategy:
- Rows (destination nodes) are dealt to 8 cores balanced by degree. Each core
  owns S=N/8 rows; sigma = (core, local slot) is the global table order.
- Node-feature table tbl[sigma(n)] = [H[0,n,:], H[1,n,:]] fp16 (128 cols),
  replicated in each core's DRAM via AllGather after every GT layer.
- Per GT layer each core processes its own rows' edges: dma_gather
  (transpose=True) pulls H[col] (feature-on-partition), PE broadcasts per-edge
  weights w = vals * filt[c, etype] into PSUM ([3,128] @ [3,n] vals-by-type),
  ACT evacuates to fp16, DVE multiplies in place and segment-sums via grouped
  tensor_reduce. Edges are pre-sorted by (stream, segment size, row) with
  compile-time quantile-padded segment sizes so one SPMD program fits all
  cores.
- int16 gather indices only reach 32768 rows, so edges split into stream A
  (sigma(col) < HALF) and B (gathered from tbl[HALF:]). Each stream reduces
  in its own row order; dma_scatter_add (DMA CCE add) combines both partial
  sums while scattering rows into the next table slice.
- Dense parts (X @ Ws einsum, Hm formula, linear+relu) run row-local; PE
  transposes move between row-major and feature-major layouts.
"""
import os
import numpy as np

import concourse.bass as bass
import concourse.tile as tile
import concourse.bacc as bacc
from concourse import mybir
from concourse.bass_utils import run_bass_kernel_spmd
from concourse.masks import make_identity

T = 3
C = 2
HID = 64
NCORES = 8
BETA = 0.5
TP = 0.8

fp16 = mybir.dt.float16
f32 = mybir.dt.float32
i16 = mybir.dt.int16


def _softmax(x):
    e = np.exp(x - x.max(axis=-1, keepdims=True))
    return e / e.sum(axis=-1, keepdims=True)


def _wrap16(a):
    n = a.shape[0]
    assert n % 16 == 0
    return np.tile(a.reshape(n // 16, 16).T, (8, 1))


class _Cfg:
    def __init__(self, n, half=32768, ch_max=6144, reps=1):
        self.N = n
        self.S = n // NCORES
        self.SPAD = -(-self.S // 128) * 128
        self.KP = self.KP = -(-self.S // 128)
        self.HALF = half
        self.CH_MAX = ch_max
        self.REPS = reps


def _plan(cfg, row, col, vals, etype):
    N, S = cfg.N, cfg.S
    deg = np.bincount(row, minlength=N)
    order = np.argsort(-deg, kind="stable")
    core_rows = [[] for _ in range(NCORES)]
    for i in range(0, N, NCORES):
        blk = order[i:i + NCORES]
        if (i // NCORES) % 2 == 0:
            for k in range(NCORES):
                core_rows[k].append(blk[k])
        else:
            for k in range(NCORES):
                core_rows[k].append(blk[NCORES - 1 - k])
    core_rows = [np.sort(np.array(r)) for r in core_rows]
    sigma = np.empty(N, np.int64)
    core_of_row = np.empty(N, np.int64)
    for k in range(NCORES):
        sigma[core_rows[k]] = k * S + np.arange(S)
        core_of_row[core_rows[k]] = k

    scol = sigma[col]
    ecore = core_of_row[row]
    stream = (scol >= cfg.HALF).astype(np.int64)
    lrow = sigma[row] - ecore * S

    plans = {}
    percore = [dict() for _ in range(NCORES)]
    for s in (0, 1):
        ds = np.zeros((NCORES, S), np.int64)
        m = stream == s
        np.add.at(ds, (ecore[m], lrow[m]), 1)
        order_s = np.argsort(ds, axis=1, kind="stable")
        dsorted = np.take_along_axis(ds, order_s, axis=1)
        DS = dsorted.max(axis=0)
        j0 = int(np.argmax(DS > 0)) if DS.any() else S
        chunks = []
        j = j0
        while j < S:
            tot = 0
            jj = j
            sizes = []
            while jj < S and tot + DS[jj] <= cfg.CH_MAX:
                tot += int(DS[jj])
                sizes.append(int(DS[jj]))
                jj += 1
            pad = (-tot) % 128
            sizes[-1] += pad
            chunks.append(np.array(sizes, np.int64))
            j = jj
        plans[s] = dict(chunks=chunks, R=S - j0, j0=j0)

        sizes_all = (np.concatenate(chunks) if chunks else np.zeros(0, np.int64))
        starts = np.concatenate([[0], np.cumsum(sizes_all)])[:-1]
        total_slots = int(sizes_all.sum())
        for k in range(NCORES):
            rank = np.empty(S, np.int64)
            rank[order_s[k]] = np.arange(S)
            mk = m & (ecore == k)
            er = rank[lrow[mk]]
            so = np.argsort(er, kind="stable")
            er = er[so]
            eidx = (scol[mk][so] - s * cfg.HALF).astype(np.int16)
            eval_ = vals[mk][so].astype(np.float16)
            et = etype[mk][so].astype(np.int64)
            cov = er - j0
            dcount = ds[k][order_s[k]]
            cum = np.concatenate([[0], np.cumsum(dcount[j0:])])[:-1]
            within = (np.arange(er.shape[0]) - cum[cov]) if er.shape[0] else np.zeros(0, np.int64)
            slot = starts[cov] + within
            a_idx = np.zeros(total_slots, np.int16)
            a_val = np.zeros(total_slots, np.float16)
            a_t = np.zeros(total_slots, np.int64)
            a_idx[slot] = eidx
            a_val[slot] = eval_
            a_t[slot] = et
            sc = order_s[k][j0:].astype(np.int16)
            sc = np.concatenate([sc, np.full((-sc.shape[0]) % 16, -1, np.int16)])
            percore[k][f"sl_idx_{s}"] = a_idx
            percore[k][f"sl_val_{s}"] = a_val
            percore[k][f"sl_t_{s}"] = a_t
            percore[k][f"sc_{s}"] = sc
    return plans, percore, core_rows


def _build_inputs(cfg, inputs, plans, percore, core_rows):
    X = np.asarray(inputs["X"], np.float32)
    filts = [_softmax(np.asarray(inputs["layerW0"], np.float32)[0]),
             _softmax(np.asarray(inputs["layerW0"], np.float32)[1]),
             _softmax(np.asarray(inputs["layerW1"], np.float32)[0]),
             _softmax(np.asarray(inputs["layerW1"], np.float32)[1])]
    cvec = np.arange(128) // 64

    def ws_pack(W):
        W = np.asarray(W, np.float32)
        return np.concatenate([W[0], W[1]], axis=1).astype(np.float16)

    chunk_list = []
    gcol = 0
    vrow = 0
    for s in (0, 1):
        soff = 0
        for sizes in plans[s]["chunks"]:
            n = int(sizes.sum())
            runs = []
            i = 0
            pos = 0
            off = 0
            while i < len(sizes):
                j = i
                while j < len(sizes) and sizes[j] == sizes[i]:
                    j += 1
                runs.append((pos, j - i, int(sizes[i]), off))
                off += (j - i) * int(sizes[i])
                pos += j - i
                i = j
            chunk_list.append(dict(s=s, n=n, gcol=gcol, runs=runs,
                                   rows=len(sizes), soff=soff))
            soff += n
            gcol += n // 16
    # pack chunks into 8 row-triples of the [24, *] vals tensor
    tri_off = [0] * 8
    for ci, c in enumerate(chunk_list):
        tri = ci % 8
        c["tri"] = tri
        c["vcol"] = tri_off[tri]
        tri_off[tri] += c["n"]
    vmax = max(tri_off)
    # lhsT blocks [24, 128] per (layer, triple): f-values at rows 3*tri..3*tri+3
    lhsTf = np.zeros((24, 4 * 8 * 128), np.float16)
    for gl in range(4):
        for tri in range(8):
            blk = np.zeros((24, 128), np.float32)
            for t in range(T):
                blk[3 * tri + t, :] = filts[gl][cvec, t]
            lhsTf[:, 128 * (8 * gl + tri):128 * (8 * gl + tri + 1)] = blk.astype(np.float16)

    in_maps = []
    for k in range(NCORES):
        gidx = np.zeros((16, gcol), np.int16)
        valsT = np.zeros((24, vmax), np.float16)
        for c in chunk_list:
            s, n, so = c["s"], c["n"], c["soff"]
            a_idx = percore[k][f"sl_idx_{s}"][so:so + n]
            a_val = percore[k][f"sl_val_{s}"][so:so + n]
            a_t = percore[k][f"sl_t_{s}"][so:so + n]
            gidx[:, c["gcol"]:c["gcol"] + n // 16] = a_idx.reshape(n // 16, 16).T
            vb = np.zeros((3, n), np.float16)
            vb[a_t, np.arange(n)] = a_val
            valsT[3 * c["tri"]:3 * c["tri"] + 3, c["vcol"]:c["vcol"] + n] = vb
        x0T = np.zeros((64, cfg.SPAD), np.float16)
        x0T[:, :cfg.S] = X[core_rows[k]].T
        in_maps.append(dict(
            gidx=np.ascontiguousarray(np.tile(gidx, (8, 1))),
            valsT=np.ascontiguousarray(valsT),
            lhsTf=lhsTf,
            ws0=ws_pack(inputs["Ws0"]), ws1=ws_pack(inputs["Ws1"]),
            linW0=np.asarray(inputs["lin_W0"], np.float32).astype(np.float16),
            linW1=np.asarray(inputs["lin_W1"], np.float32).astype(np.float16),
            linb0=np.asarray(inputs["lin_b0"], np.float32).reshape(HID, 1),
            linb1=np.asarray(inputs["lin_b1"], np.float32).reshape(HID, 1),
            sidxA=np.ascontiguousarray(_wrap16(percore[k]["sc_0"])),
            sidxB=np.ascontiguousarray(_wrap16(percore[k]["sc_1"])),
            x0T=x0T,
        ))
    meta = dict(chunk_list=chunk_list, gcol=gcol, vmax=vmax,
                RA=plans[0]["R"], RB=plans[1]["R"],
                scA_cols=in_maps[0]["sidxA"].shape[1],
                scB_cols=in_maps[0]["sidxB"].shape[1])
    return in_maps, meta


def _build_program(cfg, meta):
    chunk_list = meta["chunk_list"]
    RA, RB = meta["RA"], meta["RB"]
    N, S, SPAD, KP = cfg.N, cfg.S, cfg.SPAD, cfg.KP

    nc = bacc.Bacc("TRN2", target_bir_lowering=False, debug=False,
                   num_devices=NCORES)

    gidx = nc.dram_tensor("gidx", [128, meta["gcol"]], i16, kind="ExternalInput").ap()
    valsT = nc.dram_tensor("valsT", [24, meta["vmax"]], fp16, kind="ExternalInput").ap()
    lhsTf = nc.dram_tensor("lhsTf", [24, 4096], fp16, kind="ExternalInput").ap()
    ws0 = nc.dram_tensor("ws0", [64, 128], fp16, kind="ExternalInput").ap()
    ws1 = nc.dram_tensor("ws1", [64, 128], fp16, kind="ExternalInput").ap()
    linW0 = nc.dram_tensor("linW0", [128, 64], fp16, kind="ExternalInput").ap()
    linW1 = nc.dram_tensor("linW1", [128, 64], fp16, kind="ExternalInput").ap()
    linb0 = nc.dram_tensor("linb0", [64, 1], f32, kind="ExternalInput").ap()
    linb1 = nc.dram_tensor("linb1", [64, 1], f32, kind="ExternalInput").ap()
    sidxA = nc.dram_tensor("sidxA", [128, meta["scA_cols"]], i16, kind="ExternalInput").ap()
    sidxB = nc.dram_tensor("sidxB", [128, meta["scB_cols"]], i16, kind="ExternalInput").ap()
    x0T_d = nc.dram_tensor("x0T", [64, SPAD], fp16, kind="ExternalInput").ap()
    o_out = nc.dram_tensor("o_out", [S, HID], f32, kind="ExternalOutput").ap()

    import contextlib
    with tile.TileContext(nc) as tc:
        with contextlib.ExitStack() as _stk:
            per = _stk.enter_context(tc.tile_pool(name="per", bufs=1))
            gp = _stk.enter_context(tc.tile_pool(name="gp", bufs=2))
            wp = _stk.enter_context(tc.tile_pool(name="wp", bufs=2))
            tp = _stk.enter_context(tc.tile_pool(name="tp", bufs=2))
            hb = _stk.enter_context(tc.tile_pool(name="hb", bufs=6))
            pp = _stk.enter_context(tc.tile_pool(name="pp", bufs=4, space="PSUM"))
            pt = _stk.enter_context(tc.tile_pool(name="pt", bufs=2, space="PSUM"))
            dram = _stk.enter_context(tc.tile_pool(name="dram", bufs=1, space="DRAM"))
            t_gidx = per.tile([128, meta["gcol"]], i16)
            nc.sync.dma_start(t_gidx[:], gidx[:])
            t_vals = per.tile([24, meta["vmax"]], fp16)
            nc.sync.dma_start(t_vals[:], valsT[:])
            t_lf = per.tile([24, 4096], fp16)
            nc.sync.dma_start(t_lf[:], lhsTf[:])
            t_ws0 = per.tile([64, 128], fp16, tag="ws0")
            t_ws1 = per.tile([64, 128], fp16, tag="ws1")
            t_ws = [t_ws0, t_ws1]
            nc.sync.dma_start(t_ws[0][:], ws0[:])
            nc.sync.dma_start(t_ws[1][:], ws1[:])
            t_lw0 = per.tile([128, 64], fp16, tag="lw0")
            t_lw1 = per.tile([128, 64], fp16, tag="lw1")
            t_lw = [t_lw0, t_lw1]
            nc.sync.dma_start(t_lw[0][:], linW0[:])
            nc.sync.dma_start(t_lw[1][:], linW1[:])
            t_lb0 = per.tile([64, 1], f32, tag="lb0")
            t_lb1 = per.tile([64, 1], f32, tag="lb1")
            t_lb = [t_lb0, t_lb1]
            nc.sync.dma_start(t_lb[0][:], linb0[:])
            nc.sync.dma_start(t_lb[1][:], linb1[:])
            t_sA = per.tile([128, meta["scA_cols"]], i16)
            nc.sync.dma_start(t_sA[:], sidxA[:])
            t_sB = per.tile([128, meta["scB_cols"]], i16)
            nc.sync.dma_start(t_sB[:], sidxB[:])
            ident = per.tile([128, 128], fp16)
            make_identity(nc, ident[:])
            t_zero = per.tile([128, SPAD], fp16)
            nc.vector.memset(t_zero[:], 0.0)

            d_tbl0 = dram.tile([N, 128], fp16, tag="tbl0")
            d_tbl1 = dram.tile([N, 128], fp16, tag="tbl1")
            d_tbl = [d_tbl0, d_tbl1]
            d_slice = dram.tile([S, 128], fp16)
            d_h2 = dram.tile([SPAD, 128], fp16)

            pieces = [(o, min(512, SPAD - o)) for o in range(0, SPAD, 512)]

            def zero_dram(dt, nrows):
                full = nrows // 128
                if full:
                    nc.sync.dma_start(
                        dt[:full * 128, :].rearrange("(k p) f -> p k f", p=128),
                        t_zero[:, :full * 128].rearrange("p (k f) -> p k f", f=128))
                rem = nrows - full * 128
                if rem:
                    nc.sync.dma_start(dt[full * 128:nrows, :], t_zero[:rem, :128])

            def transpose_to_rows(src, t_rows):
                for k in range(KP):
                    ptile = pt.tile([128, 128], fp16, space="PSUM", tag="tr")
                    nc.tensor.transpose(out=ptile[:], in_=src[:, 128 * k:128 * (k + 1)],
                                        identity=ident[:])
                    nc.scalar.activation(t_rows[:, k, :], ptile[:],
                                         mybir.ActivationFunctionType.Copy)

            def write_rows_to_dram(t_rows, dt, nrows):
                full = nrows // 128
                if full:
                    nc.sync.dma_start(
                        dt[:full * 128, :].rearrange("(k p) f -> p k f", p=128),
                        t_rows[:, :full, :])
                rem = nrows - full * 128
                if rem:
                    nc.sync.dma_start(dt[full * 128:nrows, :], t_rows[:rem, full, :])

            def scatter_rows(t_rows, dt, t_sidx, R):
                if os.environ.get("GTN_SKIP_SCATTER"):
                    return
                kp_r = -(-R // 128)
                nc.gpsimd.dma_scatter_add(
                    out_ap=dt[:], in_ap=t_rows[:, :kp_r, :],
                    idxs_ap=t_sidx[:, :-(-R // 16)],
                    num_idxs=R, num_idxs_reg=R, elem_size=128)

            def spmm_layer(gl, tbl_in, houtA, houtB):
                nc.vector.memset(houtA[:], 0.0)
                nc.vector.memset(houtB[:], 0.0)
                for c in chunk_list:
                    n = c["n"]
                    t_g = gp.tile([128, 1, cfg.CH_MAX], fp16, tag="g")
                    src = tbl_in[:] if c["s"] == 0 else tbl_in[cfg.HALF:, :]
                    if os.environ.get("GTN_SKIP_GATHER"):
                        nc.vector.memset(t_g[:, :, :n], 1.0)
                    else:
                        nc.gpsimd.dma_gather(
                            out_ap=t_g[:, :, :n], in_ap=src,
                            idxs_ap=t_gidx[:, c["gcol"]:c["gcol"] + n // 16],
                            num_idxs=n, num_idxs_reg=n, elem_size=128, transpose=True, single_packet=False)
                    t_w2 = wp.tile([128, cfg.CH_MAX], fp16, tag="w2")
                    if os.environ.get("GTN_SKIP_W2"):
                        nc.vector.memset(t_w2[:, :n], 1.0)
                    else:
                      for o in range(0, n, 512):
                        L = min(512, n - o)
                        ps = pp.tile([128, 512], f32, space="PSUM", tag="mm")
                        lfo = 128 * (8 * gl + c["tri"])
                        nc.tensor.matmul(out=ps[:, :L],
                                         lhsT=t_lf[:, lfo:lfo + 128],
                                         rhs=t_vals[:, c["vcol"] + o:c["vcol"] + o + L],
                                         start=True, stop=True)
                        nc.scalar.activation(t_w2[:, o:o + L], ps[:, :L],
                                             mybir.ActivationFunctionType.Copy)
                    nc.vector.tensor_tensor(out=t_g[:, 0, :n], in0=t_g[:, 0, :n],
                                            in1=t_w2[:, :n], op=mybir.AluOpType.mult)
                    hout = houtA if c["s"] == 0 else houtB
                    base = c["pos0"]
                    with nc.allow_low_precision("fp16 segment sums"):
                        for (pos, cnt, size, off) in c["runs"]:
                            nc.vector.tensor_reduce(
                                out=hout[:, base + pos:base + pos + cnt],
                                in_=t_g[:, 0, off:off + cnt * size].rearrange(
                                    "p (r g) -> p r g", g=size),
                                axis=mybir.AxisListType.X, op=mybir.AluOpType.add)

            baseA = baseB = 0
            for c in chunk_list:
                if c["s"] == 0:
                    c["pos0"] = baseA
                    baseA += c["rows"]
                else:
                    c["pos0"] = baseB
                    baseB += c["rows"]
            assert baseA == RA and baseB == RB, (baseA, RA, baseB, RB)

            def combine_layer(houtA, houtB, dt, nrows):
                zero_dram(dt, nrows)
                rowsA = hb.tile([128, KP, 128], fp16, tag="hb")
                transpose_to_rows(houtA, rowsA)
                scatter_rows(rowsA, dt, t_sA, RA)
                rowsB = hb.tile([128, KP, 128], fp16, tag="hb")
                transpose_to_rows(houtB, rowsB)
                scatter_rows(rowsB, dt, t_sB, RB)

            def allgather(dst_tbl):
                if os.environ.get("GTN_SKIP_AG"):
                    nc.sync.dma_start(dst_tbl[:S, :], d_slice[:])
                    return
                nc.gpsimd.collective_compute(
                    "AllGather", mybir.AluOpType.bypass,
                    replica_groups=[list(range(NCORES))],
                    ins=[d_slice[:]], outs=[dst_tbl[:]])

            for _rep in range(cfg.REPS):
              t_xT = per.tile([64, SPAD], fp16, tag="xT")
              nc.sync.dma_start(t_xT[:], x0T_d[:])

              cur = 0
              for b in range(2):
                  h0fm = hb.tile([128, SPAD], fp16, tag="hb")
                  for (o, L) in pieces:
                      ps = pp.tile([128, 512], f32, space="PSUM", tag="mm")
                      nc.tensor.matmul(out=ps[:, :L], lhsT=t_ws[b][:],
                                       rhs=t_xT[:, o:o + L], start=True, stop=True)
                      nc.scalar.activation(h0fm[:, o:o + L], ps[:, :L],
                                           mybir.ActivationFunctionType.Copy)
                  h0rows = hb.tile([128, KP, 128], fp16, tag="hb")
                  transpose_to_rows(h0fm, h0rows)
                  write_rows_to_dram(h0rows, d_slice, S)
                  allgather(d_tbl[cur])

                  houtA = hb.tile([128, SPAD], fp16, tag="hb")
                  houtB = hb.tile([128, SPAD], fp16, tag="hb")
                  spmm_layer(2 * b + 0, d_tbl[cur], houtA, houtB)
                  combine_layer(houtA, houtB, d_slice, S)
                  allgather(d_tbl[1 - cur])

                  houtA2 = hb.tile([128, SPAD], fp16, tag="hb")
                  houtB2 = hb.tile([128, SPAD], fp16, tag="hb")
                  spmm_layer(2 * b + 1, d_tbl[1 - cur], houtA2, houtB2)
                  combine_layer(houtA2, houtB2, d_h2, SPAD)

                  h2rows = hb.tile([128, KP, 128], fp16, tag="hb")
                  nc.sync.dma_start(h2rows[:],
                                    d_h2[:].rearrange("(k p) f -> p k f", p=128))
                  hm = hb.tile([128, KP, 128], fp16, tag="hb")
                  nc.vector.tensor_tensor(out=hm[:], in0=h0rows[:], in1=h2rows[:],
                                          op=mybir.AluOpType.add)
                  nc.vector.tensor_scalar_max(hm[:], hm[:], 0.0)
                  h0s = hb.tile([128, KP, 128], fp16, tag="hb")
                  nc.vector.tensor_scalar_mul(h0s[:], h0rows[:], float(1.0 - TP))
                  nc.vector.scalar_tensor_tensor(out=hm[:], in0=hm[:],
                                                 scalar=float(TP * BETA), in1=h0s[:],
                                                 op0=mybir.AluOpType.mult,
                                                 op1=mybir.AluOpType.add)
                  hmT = hb.tile([128, SPAD], fp16, tag="hb")
                  for k in range(KP):
                      ptile = pt.tile([128, 128], fp16, space="PSUM", tag="tr")
                      nc.tensor.transpose(out=ptile[:], in_=hm[:, k, :], identity=ident[:])
                      nc.scalar.activation(hmT[:, 128 * k:128 * (k + 1)], ptile[:],
                                           mybir.ActivationFunctionType.Copy)
                  if b == 0:
                      outT = per.tile([64, SPAD], fp16, tag="xT")
                  else:
                      outT = hb.tile([64, SPAD], fp16, tag="hb")
                  for (o, L) in pieces:
                      ps = pp.tile([128, 512], f32, space="PSUM", tag="mm")
                      nc.tensor.matmul(out=ps[:64, :L], lhsT=t_lw[b][:],
                                       rhs=hmT[:, o:o + L], start=True, stop=True)
                      nc.scalar.activation(outT[:, o:o + L], ps[:64, :L],
                                           mybir.ActivationFunctionType.Relu,
                                           bias=t_lb[b][:], scale=1.0)
                  if b == 0:
                      t_xT = outT
                  else:
                      for k in range(KP):
                          nr = min(128, S - 128 * k)
                          if nr <= 0:
                              break
                          ptile = pt.tile([128, 128], fp16, space="PSUM", tag="tr")
                          nc.tensor.transpose(out=ptile[:, :64],
                                              in_=outT[:, 128 * k:128 * (k + 1)],
                                              identity=ident[:64, :64])
                          t_or = tp.tile([128, 64], f32, tag="orow")
                          nc.scalar.activation(t_or[:], ptile[:, :64],
                                               mybir.ActivationFunctionType.Copy)
                          nc.sync.dma_start(o_out[128 * k:128 * k + nr, :], t_or[:nr, :])

    nc.compile()
    return nc


LAST_RESULT = None


def _run(cfg, inputs, sim=False, trace=False):
    row = np.concatenate([np.asarray(inputs[f"edge_index_{t}"])[0] for t in range(T)]).astype(np.int64)
    col = np.concatenate([np.asarray(inputs[f"edge_index_{t}"])[1] for t in range(T)]).astype(np.int64)
    vals = np.concatenate([np.asarray(inputs[f"edge_value_{t}"]) for t in range(T)]).astype(np.float32)
    etype = np.repeat(np.arange(T), row.shape[0] // T)

    plans, percore, core_rows = _plan(cfg, row, col, vals, etype)
    in_maps, meta = _build_inputs(cfg, inputs, plans, percore, core_rows)
    nc = _build_program(cfg, meta)

    if sim:
        from concourse.bass_interp import MultiCoreSim
        msim = MultiCoreSim(nc, num_cores=NCORES, trace=False)
        for k, core in enumerate(msim.cores.values()):
            for name, arr in in_maps[k].items():
                core.tensor(name)[:] = arr
        msim.simulate(check_with_hw=False)
        results = [{"o_out": np.asarray(core.tensor("o_out")).copy()}
                   for core in msim.cores.values()]
    else:
        res = run_bass_kernel_spmd(nc, in_maps, core_ids=list(range(NCORES)),
                                   trace=trace)
        globals()["LAST_RESULT"] = res
        results = res.results

    out = np.empty((cfg.N, HID), np.float32)
    for k in range(NCORES):
        out[core_rows[k]] = results[k]["o_out"]
    return out


def kernel(**inputs):
    cfg = _Cfg(n=50000, half=32768, ch_max=6144)
    return _run(cfg, inputs, sim=False)

